# revision 32
# baseline (speedup 1.0000x reference)
"""Trainium2 Bass kernel: fused multi-head causal self-attention block.

Computes, for x:(B,S,H), W_qkv:(3H,H), b_qkv:(3H,), W_out:(H,H), b_out:(H,):
    qkv = x @ W_qkv.T + b_qkv ; split into q,k,v heads (NH heads, D=H/NH)
    out = softmax(causal(q k^T / sqrt(D))) v   ; merge heads
    return out @ W_out.T + b_out

Sharding over 8 NeuronCores: DP(2 batches) x TP(4 head-groups).
Core c handles batch b=c//4, head group g=c%4 (heads 4g..4g+3).

The output projection is sharded over TOKENS (not output columns): each
core computes out[512g:512(g+1), :] for its batch.  A per-head-slice
attention tile an(l, qs) = [d=128, q=512] is needed by exactly ONE peer
(the owner of token strip qs), so the communication is an AllToAll of
[128,512] tiles per local head l -- ~3.4x less fabric traffic than the
AllGather of all tiles to all peers.  The received tiles are already in
lhsT layout for the out-proj matmuls.  The host concatenates the
per-core [512, 2048] outputs along tokens.

The runtime only supports AllToAll on >4-core (mesh) groups, so the two
4-core batch groups share one 8-core AllToAll: every core writes its
tile, multiplied by per-core 0/1 batch-select masks (input data, so the
SPMD program stays identical), into BOTH batch-halves' chunk slots; the
receiver adds chunk r and chunk r+4, and the cross-batch half is zero.

All matmul operands are fp16 (PSUM accumulation is fp32); softmax
denominators and normalization stay fp32.
"""

import math

import numpy as np

import concourse.bass as bass
import concourse.mybir as mybir
import concourse.tile as tile
from concourse import bacc
from concourse.bass_utils import run_bass_kernel_spmd

FP = mybir.dt.float32
F16 = mybir.dt.float16

# Full-size problem constants.
B, S, H, NH = 2, 2048, 2048, 16
D = 128
NCORES = 8
GROUPS = 4                  # head-groups per batch (TP degree)
REPLICA_GROUPS = [[0, 1, 2, 3], [4, 5, 6, 7]]

SKEW = 2                    # attention accumulation lag (scores stay ahead)
TRACE = False               # set by test harness to capture NTFF profile
LAST_EXEC_NS = None
LAST_RESULTS = None


def build_nc(s=S, h=H, nh=NH, reps=1, ag=True):
    """Build the SPMD Bass program (identical on all 8 cores)."""
    nl = nh // GROUPS           # local heads per core
    dg = nl * D                 # per-core slice of the head dim
    scale = 1.0 / math.sqrt(D)

    nc = bacc.Bacc(
        "TRN2",
        target_bir_lowering=False,
        debug=False,
        enable_asserts=False,
        num_devices=NCORES,
    )

    # ---- I/O -----------------------------------------------------------
    xT_d = nc.dram_tensor("xT", [h, s], F16, kind="ExternalInput")
    wq_d = nc.dram_tensor("wq", [h, dg], F16, kind="ExternalInput")
    wk_d = nc.dram_tensor("wk", [h, dg], F16, kind="ExternalInput")
    wv_d = nc.dram_tensor("wv", [h, dg], F16, kind="ExternalInput")
    wo_d = nc.dram_tensor("wo", [h, h], F16, kind="ExternalInput")
    bq_d = nc.dram_tensor("bq", [128, nl], FP, kind="ExternalInput")
    bk_d = nc.dram_tensor("bk", [128, nl], FP, kind="ExternalInput")
    bv_d = nc.dram_tensor("bv", [128, dg], FP, kind="ExternalInput")
    bo_d = nc.dram_tensor("bo", [128, h], FP, kind="ExternalInput")
    mask_d = nc.dram_tensor("mask", [128, 896], F16, kind="ExternalInput")
    ones_d = nc.dram_tensor("ones", [128, 128], F16, kind="ExternalInput")
    bsel_d = nc.dram_tensor("bsel", [128, 2], FP, kind="ExternalInput")
    out_d = nc.dram_tensor("out", [s // GROUPS, h], FP, kind="ExternalOutput")

    with tile.TileContext(nc) as tc:
        with tc.tile_pool(name="const", bufs=1) as constp:
            mask_sb = constp.tile([128, 896], F16)
            ones_sb = constp.tile([128, 128], F16)
            bq_sb = constp.tile([128, nl], FP)
            bk_sb = constp.tile([128, nl], FP)
            bv_sb = constp.tile([128, dg], FP)
            bsel_sb = constp.tile([128, 2], FP)
            ones32_sb = constp.tile([128, 128], mybir.dt.float32r)
            ones_sq = ones32_sb[:, :]         # [128,128] lhsT: denominator+broadcast

            nc.sync.dma_start(bq_sb[:], bq_d[:])
            nc.sync.dma_start(bk_sb[:], bk_d[:])
            nc.sync.dma_start(mask_sb[:], mask_d[:])
            nc.sync.dma_start(ones_sb[:], ones_d[:])
            nc.sync.dma_start(bv_sb[:], bv_d[:])
            nc.sync.dma_start(bsel_sb[:], bsel_d[:])
            # f32r memset fails ISA codegen; build the f32r ones tile by
            # DVE-copying the (exactly representable) f16 ones.
            nc.vector.tensor_copy(ones32_sb[:], ones_sb[:])

            for _rep in range(reps):
                _emit_body(nc, tc, s, h, nh,
                           xT_d, wq_d, wk_d, wv_d, wo_d, bo_d, out_d,
                           bq_sb, bk_sb, bv_sb, bsel_sb,
                           mask_sb, ones_sq, scale, ag)

    nc.compile()
    return nc


def _emit_body(nc, tc, s, h, nh,
               xT_d, wq_d, wk_d, wv_d, wo_d, bo_d, out_d,
               bq_sb, bk_sb, bv_sb, bsel_sb,
               mask_sb, ones_sq, scale, ag=True):
    nl = nh // GROUPS
    dg = nl * D
    hc = h // 128               # 128-row contraction chunks
    hb_n = hc // 4              # batched (4-chunk) groups
    sq = s // 512
    st_n = s // 128             # 128-row s tiles
    with tc.tile_pool(name="qkv", bufs=1) as qkvp:
        qT = [qkvp.tile([128, s], F16, tag=f"qT{t}", name=f"qT{t}") for t in range(nl)]
        kT = [qkvp.tile([128, s], F16, tag=f"kT{t}", name=f"kT{t}") for t in range(nl)]
        vv = [qkvp.tile([128, dg], F16, tag=f"v{t}", name=f"v{t}") for t in range(st_n)]

        with tc.tile_pool(name="wqkv", bufs=1) as wqkvp, \
             tc.tile_pool(name="xres", bufs=1) as xp:
            # All projection weights loaded once, alive through both A phases.
            wq_sb = [wqkvp.tile([128, 4, dg], F16, tag=f"wq{hb}", name=f"wq{hb}") for hb in range(hb_n)]
            wk_sb = [wqkvp.tile([128, 4, dg], F16, tag=f"wk{hb}", name=f"wk{hb}") for hb in range(hb_n)]
            wv_sb = [wqkvp.tile([128, 4, dg], F16, tag=f"wv{hb}", name=f"wv{hb}") for hb in range(hb_n)]
            # x resident in SBUF fp16, loaded once: [128, chunk, tokens] per hb.
            xsb = [xp.tile([128, 4, s], F16, tag=f"x{hb}", name=f"x{hb}") for hb in range(hb_n)]
            # per-queue DMA bandwidth is ~22GB/s: split every load into
            # per-128-row-chunk DMAs so they spread across the 16 queues,
            # and split x further into per-512-token-strip pieces issued in
            # A1's consumption order (wq + x strip 0 first, wk before the
            # second half-group needs it, wv before A2).  The ~0.6us
            # per-dma_start issue cost serializes on one engine queue, so
            # spread the issuing over four otherwise-idle engine queues.
            # strictly phase-ordered rounds matching A1's consumption order,
            # issued from two engines in lockstep so transfers for later
            # strips never steal HBM/queue bandwidth from the critical
            # prefix (wq + x strip 0).
            for hb in range(hb_n):
                for c in range(4):
                    rc = slice(512 * hb + 128 * c, 512 * hb + 128 * c + 128)
                    nc.scalar.dma_start(wq_sb[hb][:, c, :], wq_d[rc, :])
                    nc.sync.dma_start(xsb[hb][:, c, 0:512], xT_d[rc, 0:512])
            for hb in range(hb_n):
                for c in range(4):
                    rc = slice(512 * hb + 128 * c, 512 * hb + 128 * c + 128)
                    nc.scalar.dma_start(wk_sb[hb][:, c, :], wk_d[rc, :])
                    nc.sync.dma_start(xsb[hb][:, c, 512:1024], xT_d[rc, 512:1024])
            for hb in range(hb_n):
                for c in range(4):
                    rc = slice(512 * hb + 128 * c, 512 * hb + 128 * c + 128)
                    nc.scalar.dma_start(wv_sb[hb][:, c, :], wv_d[rc, :])
                    nc.sync.dma_start(xsb[hb][:, c, 1024:1536], xT_d[rc, 1024:1536])
            for hb in range(hb_n):
                for c in range(4):
                    rc = slice(512 * hb + 128 * c, 512 * hb + 128 * c + 128)
                    eng = nc.scalar if c % 2 else nc.sync
                    eng.dma_start(xsb[hb][:, c, 1536:2048], xT_d[rc, 1536:2048])

            # ---- Phase A1: Q^T and K^T projections ------------------
            # contraction-OUTER emission in half-groups of 4 output tiles:
            # the first matmuls only need the hb=0 chunks of x/w (compute
            # starts while later DMAs land), and each half's activation
            # drain hides under the other half's matmuls (4 psum tags x
            # bufs=2 = 8 banks).
            with tc.tile_pool(name="psA", bufs=2, space="PSUM") as psA:
                for strip in range(sq):
                    cs = slice(512 * strip, 512 * strip + 512)
                    for half in range(2):
                        gis = list(range(4 * half, 4 * half + 4))
                        pss = {gi: psA.tile([128, 512], FP, tag=f"psqk{gi % 4}",
                                            name=f"psqk{gi}")
                               for gi in gis}
                        for hb in range(hb_n):
                            for c in range(4):
                                hh = 4 * hb + c
                                for gi in gis:
                                    w_sb = wq_sb if gi < nl else wk_sb
                                    t = gi % nl
                                    nc.tensor.matmul(
                                        pss[gi][:],
                                        w_sb[hb][:, c, 128 * t:128 * t + 128],
                                        xsb[hb][:, c, cs],
                                        start=(hh == 0), stop=(hh == hc - 1),
                                    )
                        for gi in gis:
                            t = gi % nl
                            dstT = qT if gi < nl else kT
                            bias = bq_sb if gi < nl else bk_sb
                            nc.scalar.activation(
                                dstT[t][:, cs], pss[gi][:],
                                mybir.ActivationFunctionType.Identity,
                                bias=bias[:, t:t + 1],
                            )

            # ---- Phase A2: V projection (natural [s, d] layout) -----
            with tc.tile_pool(name="psV", bufs=2, space="PSUM") as psV:
                for strip in range(sq):
                    psv = [psV.tile([128, dg], FP, tag=f"psv{sti}", name=f"psv{sti}")
                           for sti in range(4)]
                    for sti in range(4):
                        ts = slice(512 * strip + 128 * sti, 512 * strip + 128 * sti + 128)
                        for hb in range(hb_n):
                            for c in range(4):
                                hh = 4 * hb + c
                                nc.tensor.matmul(
                                    psv[sti][:],
                                    xsb[hb][:, c, ts],
                                    wv_sb[hb][:, c, :],
                                    start=(hh == 0), stop=(hh == hc - 1),
                                )
                        nc.vector.tensor_add(vv[4 * strip + sti][:], psv[sti][:], bv_sb[:])

        # ---- Phase B: attention + per-head AllToAll + token-sharded out-proj
        with tc.tile_pool(name="wop", bufs=1) as wop, \
             tc.tile_pool(name="bop", bufs=1) as bop, \
             tc.tile_pool(name="etp", bufs=8) as etp, \
             tc.tile_pool(name="atp", bufs=3) as atp, \
             tc.tile_pool(name="rbp", bufs=2) as rbp, \
             tc.tile_pool(name="oaccp", bufs=1) as oaccp, \
             tc.tile_pool(name="atsp", bufs=3) as atsp, \
             tc.tile_pool(name="outp", bufs=2) as outp, \
             tc.tile_pool(name="dramp", bufs=1, space="DRAM") as dramp, \
             tc.tile_pool(name="psS", bufs=2, space="PSUM") as psS, \
             tc.tile_pool(name="psAV", bufs=2, space="PSUM") as psAV, \
             tc.tile_pool(name="psDN", bufs=2, space="PSUM") as psDN, \
             tc.tile_pool(name="psO", bufs=2, space="PSUM") as psO:
            _emit_attention(nc, tc, s, nl, dg, sq, st_n, scale, ag,
                            qT, kT, vv, mask_sb, ones_sq, bsel_sb,
                            wo_d, bo_d, out_d,
                            wop, bop, etp, atp, rbp, oaccp, atsp, outp, dramp,
                            psS, psAV, psDN, psO)


def _emit_attention(nc, tc, s, nl, dg, sq, st_n, scale, ag,
                    qT, kT, vv, mask_sb, ones_sq, bsel_sb,
                    wo_d, bo_d, out_d,
                    wop, bop, etp, atp, rbp, oaccp, atsp, outp, dramp,
                    psS, psAV, psDN, psO):
        # out accumulator: 4 token-subtiles x full 2048 out-cols, fp32.
        oacc = [oaccp.tile([128, 2048], FP, tag=f"oacc{sti}", name=f"oacc{sti}")
                for sti in range(4)]
        bo_sb = bop.tile([128, 2048], FP, tag="bo", name="bo")
        nc.sync.dma_start(bo_sb[:], bo_d[:])

        a2a_groups = [[0, 1, 2, 3, 4, 5, 6, 7]]
        if ag:
            # Dummy collective issued at the head of the (otherwise empty)
            # gpsimd queue: pays the ~77us collective-fabric cold-start
            # during phase A instead of on the first real AllToAll.
            win = dramp.tile([8, 128, 16], F16, tag="agwarm_i", name="agwarm_i")
            wout = dramp.tile([8, 128, 16], F16, tag="agwarm_o", name="agwarm_o")
            nc.gpsimd.collective_compute(
                "AllToAll",
                mybir.AluOpType.bypass,
                replica_groups=a2a_groups,
                ins=[win.opt()],
                outs=[wout.opt()],
            )

        a2a_in = {}
        a2a_out = {}
        wo4 = {}
        at4s = {}

        def load_wo(l):
            wo4[l] = wop.tile([128, 4, 2048], F16, tag="wo", name="wo", bufs=3)
            for r in range(4):
                for q in range(4):  # 128KB pieces spread across queues;
                    # issued from gpsimd (idle in phase B) to keep the sync
                    # queue free for the latency-critical an sends.
                    nc.gpsimd.dma_start(
                        wo4[l][:, r, 512 * q:512 * q + 512],
                        wo_d[512 * l + 128 * r:512 * l + 128 * r + 128,
                             512 * q:512 * q + 512])

        def att_strip(l, qs, finish_prev=None):
            """Attention for head l, q-strip qs; an tile -> a2a_in[l][qs].

            Diagonal tiles (128*kt >= 512*qs) are narrowed to their causally
            valid column range [off, 512) and their mask ([128,128] band) +
            accumulation are deferred to the end of the strip, so the PE
            never waits on the exp->mask chain.

            The softmax denominator is accumulated on the DVE (et_sum, fp32)
            instead of burning PE cycles on a per-kt ones-matmul; a single
            f32r ones-matmul at strip end reduces over partitions and
            broadcasts.  That final matmul + normalize + send are returned
            as a `finish` callback which the caller flushes after the NEXT
            strip's first score matmul, so the PE never idles on the DVE
            accumulation chain.
            """
            qb = 512 * qs
            ps_av = psAV.tile([128, 512], FP, tag="ps_av", name="ps_av")
            # denominator + partition-broadcast fused: ones[128,128]^T @
            # et_sum puts sum_k et[k, q] in EVERY output partition.
            ps_dn = psDN.tile([128, 512], FP, tag="ps_dn", name="ps_dn")
            et_sum = rbp.tile([128, 512], mybir.dt.float32r, tag="et_sum",
                              name="et_sum")
            nk = 4 * qs + 4
            diag = [kt for kt in range(nk) if 128 * kt >= qb]
            offd = [kt for kt in range(nk) if 128 * kt < qb]
            # full-width off-diagonal tiles first; narrowed+masked diagonal
            # tiles last (their exp->mask chain hides under the 3-matmul
            # accumulation steps in between). Accumulated regions are nested
            # decreasing, so partial-region psum accumulation stays valid.
            score_order = offd + diag
            accum_order = offd + diag

            ets = {}

            def emit_scores(kt):
                off = max(0, 128 * kt - qb)
                ps_s = psS.tile([128, 512], FP, tag="ps_s", name="ps_s")
                nc.tensor.matmul(
                    ps_s[:, off:512],
                    kT[l][:, 128 * kt:128 * kt + 128],
                    qT[l][:, qb + off:qb + 512],
                    start=True, stop=True,
                )
                et = etp.tile([128, 512], F16, tag="et", name="et")
                nc.scalar.activation(
                    et[:, off:512], ps_s[:, off:512],
                    mybir.ActivationFunctionType.Exp,
                    scale=scale,
                )
                if 128 * kt >= qb:  # diagonal: mask the leading 128-col band
                    nc.vector.tensor_mul(
                        et[:, off:off + 128], et[:, off:off + 128],
                        mask_sb[:, 384:512])
                ets[kt] = et

            first = accum_order[0]

            def emit_accum(kt):
                et = ets.pop(kt)
                off = max(0, 128 * kt - qb)
                if kt == first:  # widest region: initializes all of [off,512)
                    nc.vector.tensor_copy(et_sum[:, off:512], et[:, off:512])
                else:
                    nc.vector.tensor_add(et_sum[:, off:512],
                                         et_sum[:, off:512], et[:, off:512])
                nc.tensor.matmul(
                    ps_av[:, off:512],
                    vv[kt][:, 128 * l:128 * l + 128],
                    et[:, off:512],
                    start=(kt == first), stop=(kt == accum_order[-1]),
                )

            # software-pipelined emission: accumulation trails scoring by
            # SKEW; the previous strip's finish chain flushes once this
            # strip's first score matmul is in the PE queue.
            na = 0
            for i in range(nk):
                emit_scores(score_order[i])
                if i == 0 and finish_prev is not None:
                    finish_prev()
                if i >= SKEW:
                    emit_accum(accum_order[na])
                    na += 1
            while na < nk:
                emit_accum(accum_order[na])
                na += 1

            def finish():
                nc.tensor.matmul(ps_dn[:], ones_sq, et_sum[:],
                                 start=True, stop=True)
                # normalize: an = ps_av * (1/denom); batch-masked copies go
                # to both batch-halves' chunk slots (cross-batch one is 0).
                rb_bc = rbp.tile([128, 512], FP, tag="rb_bc", name="rb_bc")
                nc.vector.reciprocal_approx_fast(rb_bc[:], ps_dn[:])
                an = atp.tile([128, 512], F16, tag="an", name="an")
                nc.vector.tensor_mul(an[:], ps_av[:], rb_bc[:])
                an0 = atp.tile([128, 512], F16, tag="an0", name="an0")
                an1 = atp.tile([128, 512], F16, tag="an1", name="an1")
                nc.vector.tensor_scalar_mul(an0[:], an[:], bsel_sb[:, 0:1])
                nc.vector.tensor_scalar_mul(an1[:], an[:], bsel_sb[:, 1:2])
                nc.sync.dma_start(a2a_in[l][qs, 0:64, :], an0[0:64, :])
                nc.sync.dma_start(a2a_in[l][qs, 64:128, :], an0[64:128, :])
                nc.sync.dma_start(a2a_in[l][qs + 4, 0:64, :], an1[0:64, :])
                nc.sync.dma_start(a2a_in[l][qs + 4, 64:128, :], an1[64:128, :])

            return finish

        def issue_a2a(l):
            """AllToAll: chunk m of a2a_in (my masked an(l, qs=m%4)) goes to
            rank m; output chunks r/r+4 sum to rank (4b+r)'s an(l, my token
            strip) = the d-chunk of global head 4r+l for my tokens, in
            ready-to-use lhsT layout."""
            a2a_out[l] = dramp.tile([8, 128, 512], F16, tag=f"a2ao{l}",
                                    name=f"a2ao{l}")
            if ag:
                nc.gpsimd.collective_compute(
                    "AllToAll",
                    mybir.AluOpType.bypass,
                    replica_groups=a2a_groups,
                    ins=[a2a_in[l].opt()],
                    outs=[a2a_out[l].opt()],
                )
            else:  # timing ablation: local copy stands in for the collective
                nc.sync.dma_start(a2a_out[l][:], a2a_in[l][:])

        def prefetch_at4(l):
            # Issued from gpsimd right after the A2A trigger: the DMAs wait
            # on the collective's completion semaphore at the gpsimd queue
            # head, where they only delay wo-loads not needed for 2 more
            # heads -- never the latency-critical an sends (sync queue).
            ata = atsp.tile([128, 4, 512], F16, tag="ata", name="ata")
            atb = atsp.tile([128, 4, 512], F16, tag="atb", name="atb")
            for r in range(4):  # parallel DMAs across queues
                nc.gpsimd.dma_start(ata[:, r, :], a2a_out[l][r])
                nc.gpsimd.dma_start(atb[:, r, :], a2a_out[l][r + 4])
            at4 = atsp.tile([128, 4, 512], F16, tag="at4", name="at4")
            for r in range(4):
                nc.vector.tensor_add(at4[:, r, :], ata[:, r, :], atb[:, r, :])
            at4s[l] = at4

        def op_quarter(l, sti):
            """Head l's out-proj contribution for token-subtile sti."""
            at4 = at4s[l]
            for n in range(4):
                cols = slice(512 * n, 512 * n + 512)
                ps_o = psO.tile([128, 512], FP, tag="ps_o", name="ps_o")
                for r in range(4):
                    nc.tensor.matmul(
                        ps_o[:],
                        at4[:, r, 128 * sti:128 * sti + 128],
                        wo4[l][:, r, cols],
                        start=(r == 0), stop=(r == 3),
                    )
                if l == 0:
                    nc.vector.tensor_add(oacc[sti][:, cols], ps_o[:], bo_sb[:, cols])
                elif l < nl - 1:
                    nc.vector.tensor_add(oacc[sti][:, cols], ps_o[:], oacc[sti][:, cols])
                else:
                    ob = outp.tile([128, 512], FP, tag="ob", name="ob")
                    nc.vector.tensor_add(ob[:], ps_o[:], oacc[sti][:, cols])
                    nc.sync.dma_start(out_d[128 * sti:128 * sti + 128, cols], ob[:])

        # Per-head schedule. Strips run longest-first (qs=3..0) so the head's
        # first strip covers collective latency.  Out-proj quarters for head
        # l run during head l+2's strips: the tail then holds two heads of
        # op work, of which the first (head nl-2) is A2A-independent cover
        # for the LAST head's AllToAll -- the PE never stalls or lets the
        # HAM clock-gate ramp down waiting on it.
        strip_order = [3, 2, 1, 0]
        load_wo(0)
        load_wo(1)

        def op_head(l):
            if l + 2 < nl:
                load_wo(l + 2)
            for sti in strip_order:
                yield op_quarter, l, sti

        pending = []
        fin = None

        for l in range(nl):
            a2a_in[l] = dramp.tile([8, 128, 512], F16, tag=f"a2ai{l}",
                                   name=f"a2ai{l}")
            if l >= 2:
                pending = list(op_head(l - 2))
            for i, qs in enumerate(strip_order):
                fin = att_strip(l, qs, finish_prev=fin)
                if pending and i > 0:
                    fn, al, asti = pending.pop(0)
                    fn(al, asti)
            fin()       # flush the last strip's normalize+sends pre-A2A
            fin = None
            issue_a2a(l)
            prefetch_at4(l)
            for fn, al, asti in pending:
                fn(al, asti)
            pending = []
        for l in (nl - 2, nl - 1):
            for fn, al, asti in op_head(l):
                fn(al, asti)


def make_inputs(x, W_qkv, b_qkv, W_out, b_out, s=S, h=H, nh=NH):
    """Host-side sharding: per-core input dicts."""
    nl = nh // GROUPS
    dg = nl * D
    x = np.ascontiguousarray(np.asarray(x, dtype=np.float32))
    W_qkv = np.asarray(W_qkv, dtype=np.float32)
    b_qkv = np.asarray(b_qkv, dtype=np.float32)
    W_out = np.asarray(W_out, dtype=np.float32)
    b_out = np.asarray(b_out, dtype=np.float32)

    # causal staircase master mask: mask[i, u] = 1 iff u >= i + 384
    uu = np.arange(896)[None, :]
    ii = np.arange(128)[:, None]
    mask = (uu >= ii + 384).astype(np.float16)
    ones = np.ones((128, 128), dtype=np.float16)

    WoT = W_out.T  # [h (d-in), h (n-out)]
    # W_out^T rows permuted to the AllToAll d-order: per local-head index l,
    # chunk r = global head 4r+l (the head held as local-head l by rank r).
    blocks = []
    for l in range(nl):
        for r in range(GROUPS):
            hh = nl * r + l
            blocks.append(WoT[D * hh:D * (hh + 1), :])
    wo = np.ascontiguousarray(
        np.concatenate(blocks, axis=0).astype(np.float16))  # [h, h] fp16
    bo = np.ascontiguousarray(np.tile(b_out[None, :], (128, 1)))  # [128, h]

    in_maps = []
    for c in range(NCORES):
        b, g = divmod(c, GROUPS)
        xT = np.ascontiguousarray(x[b].T.astype(np.float16))     # [h, s]
        wq = np.ascontiguousarray(W_qkv[dg * g:dg * (g + 1), :].T.astype(np.float16))
        wk = np.ascontiguousarray(W_qkv[h + dg * g:h + dg * (g + 1), :].T.astype(np.float16))
        wv = np.ascontiguousarray(W_qkv[2 * h + dg * g:2 * h + dg * (g + 1), :].T.astype(np.float16))
        bq = np.ascontiguousarray(
            b_qkv[dg * g:dg * (g + 1)].reshape(nl, 128).T)      # [128, nl]
        bk = np.ascontiguousarray(
            b_qkv[h + dg * g:h + dg * (g + 1)].reshape(nl, 128).T)
        bv = np.tile(b_qkv[2 * h + dg * g:2 * h + dg * (g + 1)][None, :], (128, 1))
        bsel = np.zeros((128, 2), dtype=np.float32)
        bsel[:, b] = 1.0
        in_maps.append({
            "xT": xT, "wq": wq, "wk": wk, "wv": wv, "wo": wo,
            "bq": bq, "bk": bk,
            "bv": np.ascontiguousarray(bv), "bo": bo,
            "mask": mask, "ones": ones, "bsel": bsel,
        })
    return in_maps


_NC_CACHE = {}


def _get_nc(key=(S, H, NH)):
    if key not in _NC_CACHE:
        _NC_CACHE[key] = build_nc(*key)
    return _NC_CACHE[key]


def kernel(x, W_qkv, b_qkv, W_out, b_out):
    global LAST_EXEC_NS, LAST_RESULTS
    nc = _get_nc()
    in_maps = make_inputs(x, W_qkv, b_qkv, W_out, b_out)
    res = run_bass_kernel_spmd(
        nc, in_maps, core_ids=list(range(NCORES)), trace=TRACE)
    LAST_EXEC_NS = res.exec_time_ns
    LAST_RESULTS = res
    sg = S // GROUPS
    out = np.empty((B, S, H), dtype=np.float32)
    for c in range(NCORES):
        b, g = divmod(c, GROUPS)
        out[b, sg * g:sg * (g + 1), :] = res.results[c]["out"]
    return out
